# revision 2
# baseline (speedup 1.0000x reference)
"""Trainium2 Bass kernel for nn_CrossAttention (B=8, H=W=64, D=256, M=1024).

Per-sample computation:
    out = LayerNorm(MLP(softmax(x @ ctx^T) @ ctx) + x)   over [H,W,D], no affine

Sharding: data-parallel over batch. 8 batches -> 8 NeuronCores, one batch per
core, no cross-core communication (LayerNorm reduces within a sample).

v2 dataflow: scores are computed TRANSPOSED (S^T[m, tok] = ctx @ x^T) so the
softmax numerator P^T = exp(S^T - 64) lands directly in the [m, tok] layout
that the attention-output matmul needs as rhs -- the 256 DMA xbar transposes
of v1 (the hardware bottleneck) disappear entirely.

Per-core, tok = H*W = 4096 tokens in 16 chunks of 256 (2 tiles of 128):
  P1   S^T[m-tile, 256tok] = ctxT.T @ xT    (fp32r, N=256 keeps full PE rate)
  SM   P^T = exp(S^T - 64): global shift instead of per-row max (scores are
       N(0,16^2) so |S|<100 and exp stays in range; softmax is shift-
       invariant). Normalization is DEFERRED past all matmuls.
  P2   out^T[d, tok] = sum_s ctx[s-block]^T @ P^T[s]  (bf16)
       sums[1, tok]  = sum_s ones^T @ P^T[s]          (softmax denominators,
       an M=1 matmul group since the row sum is now a partition reduction)
  MLP  h^T = W1T.T @ out^T + b1 (x) sums  (K=1 bias-extension matmul keeps
       the deferred scaling consistent); relu (scale-invariant, sums>0);
       y[tok,d] = relu_h^T.T @ W2T + b2 (x) sums  == sums * true_y
  RES  recip row 1/sums -> column layout via two K=1 f32 matmuls;
       z = y*recip + x in one DVE scalar_tensor_tensor; bn_stats per tile
  LN   bn_aggr + ones-matmul across partitions, broadcast (1/std, -mean/std)
       via K=1 matmul, apply with DVE tensor_scalar, DMA out.

Chunks are software-pipelined: P1/exp of chunk ch+1 issue before P2/MLP of
chunk ch, so the tensor engine never waits on the activation engine.
"""

import sys

sys.path.insert(0, "/opt/trn_rl_repo")

import numpy as np
import ml_dtypes

import concourse.bass as bass
import concourse.mybir as mybir
import concourse.tile as tile
from concourse import bacc
from concourse.bass_utils import run_bass_kernel_spmd

F32 = mybir.dt.float32
F32R = mybir.dt.float32r
BF16 = mybir.dt.bfloat16
AF = mybir.ActivationFunctionType
ALU = mybir.AluOpType

B, H, W, D, M = 8, 64, 64, 256, 1024
TOK = H * W                 # 4096 tokens per batch
NT = TOK // 128             # 32 token tiles
NCH = NT // 2               # 16 chunks of 2 tiles (256 tokens)
NS = M // 128               # 8 context tiles
EXP_SHIFT = -64.0           # softmax stability shift (scores ~N(0,16), |max|<100)

_CACHED = {}


def _build_program(n_reps=1):
    nc = bacc.Bacc("TRN2", target_bir_lowering=False, debug=False)

    xT_d = nc.declare_dram_parameter("xT", [2, 128, TOK], F32R, isOutput=False)
    xr_d = nc.declare_dram_parameter("xr", [TOK, D], F32, isOutput=False)
    ctxT_d = nc.declare_dram_parameter("ctxT", [2, 128, M], F32R, isOutput=False)
    ctxb_d = nc.declare_dram_parameter("ctxb", [M, D], BF16, isOutput=False)
    w1t_d = nc.declare_dram_parameter("w1t", [D, D], BF16, isOutput=False)
    w2t_d = nc.declare_dram_parameter("w2t", [D, D], BF16, isOutput=False)
    b1_d = nc.declare_dram_parameter("b1", [1, D], BF16, isOutput=False)
    b2_d = nc.declare_dram_parameter("b2", [1, D], BF16, isOutput=False)
    y_d = nc.declare_dram_parameter("y", [TOK, D], F32, isOutput=True)

    with tile.TileContext(nc) as tc:
        with (
            tc.tile_pool(name="const", bufs=1) as cpool,
            tc.tile_pool(name="xin", bufs=3) as xin_pool,
            tc.tile_pool(name="pt", bufs=2) as pt_pool,
            tc.tile_pool(name="mid", bufs=2) as mid_pool,
            tc.tile_pool(name="outp", bufs=3) as out_pool,
            tc.tile_pool(name="psS", bufs=4, space="PSUM") as psS,
            tc.tile_pool(name="psMid", bufs=2, space="PSUM") as psMid,
            tc.tile_pool(name="psY", bufs=2, space="PSUM") as psY,
        ):
            # ---- persistent SBUF state ----
            ctxT_sb = cpool.tile([128, 2, M], F32R)
            xT_sb = cpool.tile([128, 2, TOK], F32R)
            ctxm_sb = cpool.tile([128, NS, D], BF16)
            w1t_sb = cpool.tile([128, 2, D], BF16)
            w2t_sb = cpool.tile([128, 2, D], BF16)
            b1_sb = cpool.tile([1, D], BF16)
            b2_sb = cpool.tile([1, D], BF16)
            ones_bf = cpool.tile([128, 1], BF16)
            one_f = cpool.tile([1, 1], F32)
            ones_row_f = cpool.tile([1, 128], F32)
            ones_col_f = cpool.tile([128, 1], F32)
            eps_sb = cpool.tile([1, 1], F32)
            shift_sb = cpool.tile([128, 1], F32)
            z_sb = cpool.tile([128, NT, D], F32)
            stats_sb = cpool.tile([128, NT, 6], F32)

            nc.vector.memset(ones_bf, 1.0)
            nc.vector.memset(one_f, 1.0)
            nc.vector.memset(ones_row_f, 1.0)
            nc.vector.memset(ones_col_f, 1.0)
            nc.vector.memset(eps_sb, 1e-5)
            nc.vector.memset(shift_sb, EXP_SHIFT)

            # ---- input loads, ordered by first use, spread over rings ----
            nc.sync.dma_start(out=ctxT_sb[:, 0, :], in_=ctxT_d[0])
            nc.scalar.dma_start(out=ctxT_sb[:, 1, :], in_=ctxT_d[1])
            for q in range(4):
                nc.scalar.dma_start(
                    out=xT_sb[:, 0, q * 1024 : (q + 1) * 1024],
                    in_=xT_d[0][:, q * 1024 : (q + 1) * 1024],
                )
                nc.sync.dma_start(
                    out=xT_sb[:, 1, q * 1024 : (q + 1) * 1024],
                    in_=xT_d[1][:, q * 1024 : (q + 1) * 1024],
                )
                if q == 0:
                    # needed by chunk 0's P2/MLP before xT q1 is touched
                    nc.gpsimd.dma_start(
                        out=ctxm_sb, in_=ctxb_d.rearrange("(s p) d -> p s d", p=128)
                    )
                    nc.gpsimd.dma_start(
                        out=w1t_sb, in_=w1t_d.rearrange("(k p) j -> p k j", p=128)
                    )
                    nc.gpsimd.dma_start(
                        out=w2t_sb, in_=w2t_d.rearrange("(k p) j -> p k j", p=128)
                    )
                    nc.gpsimd.dma_start(out=b1_sb, in_=b1_d[:, :])
                    nc.gpsimd.dma_start(out=b2_sb, in_=b2_d[:, :])

            def issue_p1(ch):
                """Scores + exp for chunk ch -> P^T tile [128, NS, 256] bf16."""
                pt_c = pt_pool.tile([128, NS, 256], BF16, tag="pt")
                c0 = ch * 256
                for p in range(NS // 2):          # pairs of m-tiles
                    S = psS.tile([128, 2, 256], F32, tag="S")
                    for sl in range(2):
                        s = 2 * p + sl
                        for kk in range(2):
                            nc.tensor.matmul(
                                S[:, sl, :],
                                lhsT=ctxT_sb[:, kk, s * 128 : (s + 1) * 128],
                                rhs=xT_sb[:, kk, c0 : c0 + 256],
                                start=(kk == 0),
                                stop=(kk == 1),
                            )
                    nc.scalar.activation(
                        pt_c[:, 2 * p : 2 * p + 2, :],
                        S,
                        AF.Exp,
                        bias=shift_sb,
                        scale=1.0,
                    )
                return pt_c

            def issue_tail(ch, pt_c, x_sb):
                """Attention-out, MLP, residual+stats for chunk ch."""
                # P2: out^T[d, tok] = sum_s ctx[s]^T-block @ P^T[s]
                outT_ps = psMid.tile([128, 2, 256], F32, tag="mid")
                for dh in range(2):
                    for s in range(NS):
                        nc.tensor.matmul(
                            outT_ps[:, dh, :],
                            lhsT=ctxm_sb[:, s, dh * 128 : (dh + 1) * 128],
                            rhs=pt_c[:, s, :],
                            start=(s == 0),
                            stop=(s == NS - 1),
                        )
                # softmax denominators: sums[1, tok] = ones^T @ P^T
                srow_ps = psMid.tile([128, 256], F32, tag="mid")
                for s in range(NS):
                    nc.tensor.matmul(
                        srow_ps[0:1, :],
                        lhsT=ones_bf,
                        rhs=pt_c[:, s, :],
                        start=(s == 0),
                        stop=(s == NS - 1),
                    )
                outT_sb = mid_pool.tile([128, 2, 256], BF16, tag="outT")
                nc.vector.tensor_copy(outT_sb, outT_ps)
                srow_sb = mid_pool.tile([1, 256], BF16, tag="srow")
                nc.vector.tensor_copy(srow_sb, srow_ps[0:1, :])
                rrow_sb = mid_pool.tile([1, 256], F32, tag="rrow")
                nc.vector.reciprocal(rrow_sb, srow_ps[0:1, :])

                # MLP1: h^T[j, tok] = W1T.T @ out^T + b1 (x) sums_row
                hT_ps = psMid.tile([128, 2, 256], F32, tag="mid")
                for jh in range(2):
                    for kk in range(2):
                        nc.tensor.matmul(
                            hT_ps[:, jh, :],
                            lhsT=w1t_sb[:, kk, jh * 128 : (jh + 1) * 128],
                            rhs=outT_sb[:, kk, :],
                            start=(kk == 0),
                            stop=False,
                        )
                    nc.tensor.matmul(
                        hT_ps[:, jh, :],
                        lhsT=b1_sb[0:1, jh * 128 : (jh + 1) * 128],
                        rhs=srow_sb,
                        start=False,
                        stop=True,
                    )
                # 1/sums into column layout: rcol[128, tl] = rrow-seg^T
                rcol_ps = psY.tile([128, 2], F32, tag="y")
                for tl in range(2):
                    nc.tensor.matmul(
                        rcol_ps[:, tl : tl + 1],
                        lhsT=rrow_sb[0:1, tl * 128 : (tl + 1) * 128],
                        rhs=one_f,
                        start=True,
                        stop=True,
                    )
                relu_sb = mid_pool.tile([128, 2, 256], BF16, tag="relu")
                nc.vector.tensor_scalar_max(relu_sb, hT_ps, 0.0)
                rcol_sb = mid_pool.tile([128, 2], F32, tag="rcol")
                nc.vector.tensor_copy(rcol_sb, rcol_ps)

                # MLP2 per tile: y[tok, d] = relu_h^T.T @ W2T + b2 (x) sums_row
                for tl in range(2):
                    t = ch * 2 + tl
                    y_ps = psY.tile([128, D], F32, tag="y")
                    for jh in range(2):
                        nc.tensor.matmul(
                            y_ps,
                            lhsT=relu_sb[:, jh, tl * 128 : (tl + 1) * 128],
                            rhs=w2t_sb[:, jh, :],
                            start=(jh == 0),
                            stop=False,
                        )
                    nc.tensor.matmul(
                        y_ps,
                        lhsT=srow_sb[0:1, tl * 128 : (tl + 1) * 128],
                        rhs=b2_sb,
                        start=False,
                        stop=True,
                    )
                    # z = y * (1/sums) + x, then per-tile stats
                    nc.vector.scalar_tensor_tensor(
                        z_sb[:, t, :],
                        y_ps,
                        rcol_sb[:, tl : tl + 1],
                        x_sb[:, tl, :],
                        op0=ALU.mult,
                        op1=ALU.add,
                    )
                    nc.vector.bn_stats(stats_sb[:, t, :], z_sb[:, t, :])

            for _rep in range(n_reps):
                pt_cur = issue_p1(0)
                x_cur = xin_pool.tile([128, 2, D], F32, tag="x")
                nc.gpsimd.dma_start(
                    out=x_cur,
                    in_=xr_d[0:256, :].rearrange("(c p) d -> p c d", p=128),
                )
                for ch in range(NCH):
                    if ch + 1 < NCH:
                        pt_nxt = issue_p1(ch + 1)
                        x_nxt = xin_pool.tile([128, 2, D], F32, tag="x")
                        nc.gpsimd.dma_start(
                            out=x_nxt,
                            in_=xr_d[(ch + 1) * 256 : (ch + 2) * 256, :].rearrange(
                                "(c p) d -> p c d", p=128
                            ),
                        )
                    issue_tail(ch, pt_cur, x_cur)
                    if ch + 1 < NCH:
                        pt_cur, x_cur = pt_nxt, x_nxt

                # ---- LayerNorm epilogue ----
                mv = cpool.tile([128, 2], F32)
                nc.vector.bn_aggr(mv, stats_sb)
                pack = cpool.tile([128, 2], F32)
                nc.vector.tensor_copy(pack[:, 0:1], mv[:, 0:1])
                nc.vector.tensor_mul(pack[:, 1:2], mv[:, 0:1], mv[:, 0:1])
                nc.vector.tensor_add(pack[:, 1:2], pack[:, 1:2], mv[:, 1:2])
                # cross-partition sums: [1, 2] = ones_col.T @ pack
                st_ps = psY.tile([1, 2], F32, tag="y")
                nc.tensor.matmul(st_ps, lhsT=ones_col_f, rhs=pack, start=True, stop=True)
                sc = cpool.tile([1, 4], F32)
                nc.vector.tensor_scalar_mul(sc[0:1, 0:1], st_ps[0:1, 0:1], 1.0 / 128.0)
                nc.vector.tensor_scalar_mul(sc[0:1, 1:2], st_ps[0:1, 1:2], 1.0 / 128.0)
                nc.vector.tensor_mul(sc[0:1, 2:3], sc[0:1, 0:1], sc[0:1, 0:1])
                nc.vector.tensor_sub(sc[0:1, 2:3], sc[0:1, 1:2], sc[0:1, 2:3])
                nc.scalar.activation(
                    sc[0:1, 2:3], sc[0:1, 2:3], AF.Sqrt, bias=eps_sb[0:1, 0:1]
                )
                nc.vector.reciprocal(sc[0:1, 2:3], sc[0:1, 2:3])
                nc.vector.tensor_mul(sc[0:1, 3:4], sc[0:1, 0:1], sc[0:1, 2:3])
                nc.vector.tensor_scalar_mul(sc[0:1, 3:4], sc[0:1, 3:4], -1.0)
                # broadcast (rstd, -mean*rstd) to all partitions
                bc_ps = psY.tile([128, 2], F32, tag="y")
                nc.tensor.matmul(
                    bc_ps, lhsT=ones_row_f, rhs=sc[0:1, 2:4], start=True, stop=True
                )
                bc_sb = cpool.tile([128, 2], F32)
                nc.vector.tensor_copy(bc_sb, bc_ps)

                # ---- apply + writeback, 4 tiles per DMA ----
                for g in range(NT // 4):
                    o_sb = out_pool.tile([128, 4, D], F32, tag="o")
                    veng = nc.vector if g % 2 == 0 else nc.gpsimd
                    veng.tensor_scalar(
                        o_sb,
                        z_sb[:, g * 4 : (g + 1) * 4, :],
                        scalar1=bc_sb[:, 0:1],
                        scalar2=bc_sb[:, 1:2],
                        op0=ALU.mult,
                        op1=ALU.add,
                    )
                    oeng = [nc.scalar, nc.sync, nc.gpsimd][g % 3]
                    oeng.dma_start(
                        out=y_d[g * 512 : (g + 1) * 512, :].rearrange(
                            "(c p) d -> p c d", p=128
                        ),
                        in_=o_sb,
                    )

    nc.finalize()
    return nc


def _get_program(n_reps=1):
    key = ("nc", n_reps)
    if key not in _CACHED:
        _CACHED[key] = _build_program(n_reps)
    return _CACHED[key]


def _make_in_maps(inputs):
    x = np.ascontiguousarray(np.asarray(inputs["x"], dtype=np.float32))
    context = np.ascontiguousarray(np.asarray(inputs["context"], dtype=np.float32))
    W1 = np.asarray(inputs["W1"], dtype=np.float32)
    b1 = np.asarray(inputs["b1"], dtype=np.float32)
    W2 = np.asarray(inputs["W2"], dtype=np.float32)
    b2 = np.asarray(inputs["b2"], dtype=np.float32)

    bf = ml_dtypes.bfloat16
    w1t = np.ascontiguousarray(W1.T).astype(bf)          # [d_in, j]
    w2t = np.ascontiguousarray(W2.T).astype(bf)          # [j, d_out]
    b1r = np.ascontiguousarray(b1.reshape(1, D)).astype(bf)
    b2r = np.ascontiguousarray(b2.reshape(1, D)).astype(bf)

    in_maps = []
    for b in range(B):
        xf = x[b].reshape(TOK, D)
        xT = np.ascontiguousarray(xf.T).reshape(2, 128, TOK)
        ctxT = np.ascontiguousarray(context[b].T).reshape(2, 128, M)
        ctxb = context[b].astype(bf)
        in_maps.append(
            {
                "xT": xT,
                "xr": xf,
                "ctxT": ctxT,
                "ctxb": ctxb,
                "w1t": w1t,
                "w2t": w2t,
                "b1": b1r,
                "b2": b2r,
            }
        )
    return in_maps


def kernel(**inputs):
    in_maps = _make_in_maps(inputs)
    nc = _get_program()
    res = run_bass_kernel_spmd(nc, in_maps, core_ids=list(range(B)))
    out = np.stack([res.results[b]["y"].reshape(H, W, D) for b in range(B)])
    return out.astype(np.float32)


if __name__ == "__main__":
    rng = np.random.default_rng(0)
    ins = {
        "x": rng.standard_normal((B, H, W, D), dtype=np.float32),
        "context": rng.standard_normal((B, M, D), dtype=np.float32),
        "W1": rng.standard_normal((D, D), dtype=np.float32) / 16.0,
        "b1": rng.standard_normal(D, dtype=np.float32) * 0.02,
        "W2": rng.standard_normal((D, D), dtype=np.float32) / 16.0,
        "b2": rng.standard_normal(D, dtype=np.float32) * 0.02,
    }
    out = kernel(**ins)
    print("ran:", out.shape, out.dtype)
